# revision 1
# baseline (speedup 1.0000x reference)
"""Trainium2 Bass kernel for CausalSelfAttention (GQA + QK-RMSNorm + RoPE).

Problem shapes (hardcoded): B=2, S=2048, D=2048, H=16, KVH=4, HD=128.

Sharding: 8 cores = 2 batches x 4 kv-head groups. Core c handles batch
b = c // 4 and kv-group g = c % 4 (q-heads 4g..4g+3, kv head g).  Each core
computes its 4 heads end-to-end plus a partial output projection over its
512 columns of Wproj's input dim; the host sums the 4 partials per batch.

On-chip dataflow per core:
  Phase 1: stream xT, GEMM Q/K/V in [s, e] layout (fp32r), QK-RMSNorm +
           RoPE + gain on DVE/ACT, PE-transpose Q/K to [hd, s] layout.
  Phase 2: causal attention per (q-block 512, head): scores computed
           TRANSPOSED [k, q] so the softmax denominator comes from a
           ones-matmul (which also broadcasts it across partitions); exp on
           ACT; diagonal tiles masked with a triangular mask; PV matmul
           accumulates attnout^T [hd, q] over k-tiles in PSUM.
  Phase 3: partial out^T [e, s] = Wproj_slice^T-chunks @ y^T, DMA to HBM.
"""

import numpy as np

B, S, D = 2, 2048, 2048
H, KVH = 16, 4
HD = D // H            # 128
NH = H // KVH          # 4 heads per core
P = 128
ST = S // P            # 16 s-tiles
DT = D // P            # 16 d-tiles
FT = NH * HD // P      # 4 f-tiles (proj contraction per core)
QB = 512               # q-block width in phase 2
NQB = S // QB          # 4
SBW = 256              # phase-1 x DMA block width (s columns)
ROPE_BASE = 10000.0
EPS = 1e-6

_CACHE = {}


def _build_nc():
    from contextlib import ExitStack

    import concourse.mybir as mybir
    import concourse.tile as tile
    from concourse import bacc

    f32 = mybir.dt.float32
    f32r = mybir.dt.float32r
    AF = mybir.ActivationFunctionType
    MUL = mybir.AluOpType.mult
    ADD = mybir.AluOpType.add

    nc = bacc.Bacc("TRN2", target_bir_lowering=False, debug=False, num_devices=8)

    xT = nc.dram_tensor("xT", [D, S], f32r, kind="ExternalInput").ap()
    wqT = nc.dram_tensor("wqT", [D, NH * HD], f32r, kind="ExternalInput").ap()
    wkvT = nc.dram_tensor("wkvT", [D, 2 * HD], f32r, kind="ExternalInput").ap()
    wpT = nc.dram_tensor("wpT", [NH * HD, D], f32r, kind="ExternalInput").ap()
    cos2 = nc.dram_tensor("cos2", [S, HD], f32, kind="ExternalInput").ap()
    sin2 = nc.dram_tensor("sin2", [S, HD], f32, kind="ExternalInput").ap()
    qg4 = nc.dram_tensor("qg4", [P, NH], f32, kind="ExternalInput").ap()
    tri = nc.dram_tensor("tri", [P, P], f32r, kind="ExternalInput").ap()
    onesd = nc.dram_tensor("onesd", [P, P], f32r, kind="ExternalInput").ap()
    ident = nc.dram_tensor("ident", [P, P], f32, kind="ExternalInput").ap()
    outT = nc.dram_tensor("outT", [D, S], f32, kind="ExternalOutput").ap()

    with tile.TileContext(nc) as tc:
        with ExitStack() as octx:
            const = octx.enter_context(tc.tile_pool(name="const", bufs=1))
            big = octx.enter_context(tc.tile_pool(name="big", bufs=1))

            # ---- persistent stores ----
            QT = big.tile([P, NH, S], f32r)   # q^T per head: [hd, h, s]
            KT = big.tile([P, S], f32r)       # k^T: [hd, s]
            VS = big.tile([P, ST, HD], f32r)  # v: [s-part, s-tile, hd]
            YT = big.tile([P, NH, S], f32r)   # attn out^T per head: [hd, h, s]

            # =========================== Phase 1 ===========================
            with ExitStack() as ctx1:
                wpool = ctx1.enter_context(tc.tile_pool(name="wpool", bufs=1))
                xpool = ctx1.enter_context(tc.tile_pool(name="xpool", bufs=2))
                stq = ctx1.enter_context(tc.tile_pool(name="stq", bufs=2))
                stk = ctx1.enter_context(tc.tile_pool(name="stk", bufs=2))
                sml = ctx1.enter_context(tc.tile_pool(name="sml", bufs=2))
                ps_q = ctx1.enter_context(
                    tc.tile_pool(name="ps_q", bufs=2, space="PSUM"))
                ps_kv = ctx1.enter_context(
                    tc.tile_pool(name="ps_kv", bufs=2, space="PSUM"))
                ps_tr = ctx1.enter_context(
                    tc.tile_pool(name="ps_tr", bufs=3, space="PSUM"))

                xTr = xT.rearrange("(dt p) s -> p dt s", p=P)
                wqTr = wqT.rearrange("(dt p) e -> p dt e", p=P)
                wkvTr = wkvT.rearrange("(dt p) e -> p dt e", p=P)

                # Chunked DMAs so the first matmuls' deps land early.
                def load_xblk(sb):
                    t = xpool.tile([P, DT, SBW], f32r, tag="xblk", name="xblk")
                    for c in range(0, DT, 2):
                        nc.sync.dma_start(
                            t[:, c:c + 2, :],
                            xTr[:, c:c + 2, sb * SBW:(sb + 1) * SBW])
                    return t

                xblk_next = load_xblk(0)

                WQ = wpool.tile([P, DT, NH * HD], f32r)
                WKV = wpool.tile([P, DT, 2 * HD], f32r)
                for c in range(0, DT, 4):
                    nc.sync.dma_start(WKV[:, c:c + 4, :], wkvTr[:, c:c + 4, :])
                for c in range(0, DT, 2):
                    nc.sync.dma_start(WQ[:, c:c + 2, :], wqTr[:, c:c + 2, :])

                # ---- constants (needed a few microseconds in) ----
                cos_t = const.tile([P, ST, HD], f32)
                sin_t = const.tile([P, ST, HD], f32)
                cos2r = cos2.rearrange("(st p) c -> p st c", p=P)
                sin2r = sin2.rearrange("(st p) c -> p st c", p=P)
                qg_t = const.tile([P, NH], f32)
                nc.sync.dma_start(qg_t[:], qg4)
                tri_t = const.tile([P, P], f32r)
                nc.sync.dma_start(tri_t[:], tri)
                ones_t = const.tile([P, P], f32r)
                nc.sync.dma_start(ones_t[:], onesd)
                id_t = const.tile([P, P], f32)
                nc.sync.dma_start(id_t[:], ident)
                idr_t = const.tile([P, P], f32r)
                nc.sync.dma_start(idr_t[:], ident.bitcast(f32r))
                eps_t = const.tile([P, 1], f32)
                nc.vector.memset(eps_t[:], EPS)

                nsb = SBW // P
                for sb in range(S // SBW):
                    xblk = xblk_next
                    if sb + 1 < S // SBW:
                        xblk_next = load_xblk(sb + 1)
                    nc.sync.dma_start(cos_t[:, sb * nsb:(sb + 1) * nsb, :],
                                      cos2r[:, sb * nsb:(sb + 1) * nsb, :])
                    nc.sync.dma_start(sin_t[:, sb * nsb:(sb + 1) * nsb, :],
                                      sin2r[:, sb * nsb:(sb + 1) * nsb, :])
                    for jj in range(SBW // P):
                        st = sb * (SBW // P) + jj
                        xs = xblk[:, :, jj * P:(jj + 1) * P]

                        psq = ps_q.tile([P, NH * HD], f32)
                        for dt in range(DT):
                            nc.tensor.matmul(
                                psq[:], xs[:, dt],
                                WQ[:, dt],
                                start=(dt == 0), stop=(dt == DT - 1))
                        pskv = ps_kv.tile([P, 2 * HD], f32)
                        for dt in range(DT):
                            nc.tensor.matmul(
                                pskv[:], xs[:, dt],
                                WKV[:, dt],
                                start=(dt == 0), stop=(dt == DT - 1))

                        # V straight to its store
                        nc.scalar.copy(VS[:, st], pskv[:, HD:2 * HD])

                        # -- Q rmsnorm + rope + gain --
                        psq3 = psq[:].rearrange("p (h c) -> p h c", h=NH)
                        ssq = sml.tile([P, NH], f32, tag="ssq")
                        sqscr = stq.tile([P, NH, HD], f32, tag="qa")
                        for h in range(NH):
                            nc.scalar.activation(sqscr[:, h], psq3[:, h], AF.Square,
                                                 accum_out=ssq[:, h:h + 1])
                        msq = sml.tile([P, NH], f32, tag="msq")
                        nc.scalar.activation(msq[:], ssq[:], AF.Sqrt,
                                             bias=eps_t[:], scale=1.0 / HD)
                        rsq = sml.tile([P, NH], f32, tag="rsq")
                        nc.vector.reciprocal(rsq[:], msq[:])
                        rsg = sml.tile([P, NH], f32, tag="rsg")
                        nc.vector.tensor_tensor(rsg[:], rsq[:], qg_t[:], MUL)

                        qn = stq.tile([P, NH, HD], f32, tag="qn")
                        nc.vector.tensor_tensor(
                            qn[:], psq3, rsg[:, :, None].to_broadcast([P, NH, HD]),
                            MUL)
                        qa = stq.tile([P, NH, HD], f32, tag="qa")
                        nc.vector.tensor_tensor(
                            qa[:], qn[:],
                            cos_t[:, st:st + 1, :].to_broadcast([P, NH, HD]), MUL)
                        qb = stq.tile([P, NH, HD], f32, tag="qb")
                        nc.vector.tensor_tensor(
                            qb[:, :, 0:HD // 2], qn[:, :, HD // 2:HD],
                            sin_t[:, st:st + 1, 0:HD // 2].to_broadcast(
                                [P, NH, HD // 2]), MUL)
                        nc.vector.tensor_tensor(
                            qb[:, :, HD // 2:HD], qn[:, :, 0:HD // 2],
                            sin_t[:, st:st + 1, HD // 2:HD].to_broadcast(
                                [P, NH, HD // 2]), MUL)
                        qrot = stq.tile([P, NH, HD], f32, tag="qn")
                        nc.vector.tensor_tensor(qrot[:], qa[:], qb[:], ADD)

                        # -- K rmsnorm + rope --
                        ssk = sml.tile([P, 1], f32, tag="ssk")
                        skscr = stk.tile([P, HD], f32, tag="ka")
                        nc.scalar.activation(skscr[:], pskv[:, 0:HD], AF.Square,
                                             accum_out=ssk[:])
                        msk = sml.tile([P, 1], f32, tag="msk")
                        nc.scalar.activation(msk[:], ssk[:], AF.Sqrt,
                                             bias=eps_t[:], scale=1.0 / HD)
                        rsk = sml.tile([P, 1], f32, tag="rsk")
                        nc.vector.reciprocal(rsk[:], msk[:])

                        kn = stk.tile([P, HD], f32, tag="kn")
                        nc.vector.tensor_tensor(
                            kn[:], pskv[:, 0:HD], rsk[:].to_broadcast([P, HD]), MUL)
                        ka = stk.tile([P, HD], f32, tag="ka")
                        nc.gpsimd.tensor_tensor(ka[:], kn[:], cos_t[:, st], MUL)
                        kb = stk.tile([P, HD], f32, tag="kb")
                        nc.gpsimd.tensor_tensor(
                            kb[:, 0:HD // 2], kn[:, HD // 2:HD],
                            sin_t[:, st, 0:HD // 2], MUL)
                        nc.gpsimd.tensor_tensor(
                            kb[:, HD // 2:HD], kn[:, 0:HD // 2],
                            sin_t[:, st, HD // 2:HD], MUL)
                        krot = stk.tile([P, HD], f32, tag="kn")
                        nc.gpsimd.tensor_tensor(krot[:], ka[:], kb[:], ADD)

                        # -- transposes into QT / KT --
                        for h in range(NH):
                            ptr = ps_tr.tile([P, P], f32, tag="tr")
                            nc.tensor.transpose(ptr[:], qrot[:, h], id_t[:])
                            if h % 2 == 0:
                                nc.scalar.copy(QT[:, h, st * P:(st + 1) * P], ptr[:])
                            else:
                                nc.vector.tensor_copy(
                                    QT[:, h, st * P:(st + 1) * P], ptr[:])
                        ptrk = ps_tr.tile([P, P], f32, tag="tr")
                        nc.tensor.transpose(ptrk[:], krot[:], id_t[:])
                        nc.scalar.copy(KT[:, st * P:(st + 1) * P], ptrk[:])

            # ======================= Phases 2 and 3 ========================
            with ExitStack() as ctx2:
                wp2 = ctx2.enter_context(tc.tile_pool(name="wp2", bufs=1))
                ostage = ctx2.enter_context(tc.tile_pool(name="ostage", bufs=6))

                WP = wp2.tile([P, FT, D], f32r)
                nc.sync.dma_start(WP[:], wpT.rearrange("(ft p) e -> p ft e", p=P))

                # ------------------ Phases 2 + 3 interleaved ---------------
                with ExitStack() as ctx2b:
                    expool = ctx2b.enter_context(
                        tc.tile_pool(name="expool", bufs=12))
                    recpool = ctx2b.enter_context(
                        tc.tile_pool(name="recpool", bufs=3))
                    ps_s = ctx2b.enter_context(
                        tc.tile_pool(name="ps_s", bufs=4, space="PSUM"))
                    ps_o = ctx2b.enter_context(
                        tc.tile_pool(name="ps_o", bufs=1, space="PSUM"))
                    ps_d = ctx2b.enter_context(
                        tc.tile_pool(name="ps_d", bufs=1, space="PSUM"))
                    ps_p3 = ctx2b.enter_context(
                        tc.tile_pool(name="ps_p3", bufs=2, space="PSUM"))

                    for qb in range(NQB):
                        for h in range(NH):
                            oT = ps_o.tile([P, QB], f32)
                            den = ps_d.tile([P, QB], f32)
                            nk = NH * qb + NH
                            grp_start, grp_sum, grp_n = None, None, 0
                            for kt in range(nk):
                                j = kt - NH * qb  # >= 0 on diagonal tiles
                                q0 = P * j if j >= 0 else 0
                                ps = ps_s.tile([P, QB], f32)
                                nc.tensor.matmul(
                                    ps[:, q0:QB],
                                    KT[:, kt * P:(kt + 1) * P],
                                    QT[:, h, qb * QB + q0:(qb + 1) * QB],
                                    start=True, stop=True)
                                ex = expool.tile([P, QB], f32r, tag="ex")
                                if j >= 1:
                                    nc.gpsimd.memset(
                                        ex[:, 0:q0].bitcast(f32), 0.0)
                                nc.scalar.activation(
                                    ex[:, q0:QB], ps[:, q0:QB], AF.Exp)
                                if j >= 0:
                                    nc.vector.tensor_tensor(
                                        ex[:, q0:q0 + P], ex[:, q0:q0 + P],
                                        tri_t[:], MUL)
                                nc.tensor.matmul(
                                    oT[:, q0:QB], VS[:, kt], ex[:, q0:QB],
                                    start=(kt == 0), stop=(kt == nk - 1))
                                # denominator: running-sum groups of full tiles
                                # on DVE so the ones-matmul runs once per 4
                                # k-tiles on PE (the bottleneck engine)
                                if j < 0:
                                    if grp_sum is None:
                                        grp_start, grp_sum, grp_n = kt, ex, 1
                                    else:
                                        ns = expool.tile(
                                            [P, QB], f32r, tag="exs")
                                        nc.vector.tensor_tensor(
                                            ns[:], grp_sum[:], ex[:], ADD)
                                        grp_sum = ns
                                        grp_n += 1
                                    if grp_n == 4 or kt + 1 >= NH * qb:
                                        nc.tensor.matmul(
                                            den[:], ones_t[:], grp_sum[:],
                                            start=(grp_start == 0),
                                            stop=False)
                                        grp_sum, grp_n = None, 0
                                else:
                                    nc.tensor.matmul(
                                        den[:, q0:QB], ones_t[:], ex[:, q0:QB],
                                        start=(kt == 0), stop=(kt == nk - 1))
                            rec = recpool.tile([P, QB], f32, tag="rec")
                            nc.vector.reciprocal_approx_fast(rec[:], den[:])
                            nc.vector.tensor_tensor(
                                YT[:, h, qb * QB:(qb + 1) * QB], oT[:], rec[:],
                                MUL)

                        # phase-3 for this s-block (= qb) fills PE gaps
                        sb3 = qb
                        for et in range(DT):
                            po = ps_p3.tile([P, QB], f32)
                            for ft in range(FT):
                                nc.tensor.matmul(
                                    po[:],
                                    WP[:, ft, et * P:(et + 1) * P],
                                    YT[:, ft, sb3 * QB:(sb3 + 1) * QB],
                                    start=(ft == 0), stop=(ft == FT - 1))
                            ob = ostage.tile([P, QB], f32)
                            if et % 2 == 0:
                                nc.scalar.copy(ob[:], po[:])
                            else:
                                nc.vector.tensor_copy(ob[:], po[:])
                            nc.sync.dma_start(
                                outT[et * P:(et + 1) * P,
                                     sb3 * QB:(sb3 + 1) * QB], ob[:])

    nc.compile()
    return nc


def _host_inputs(x, Wq, Wk, Wv, Wproj, q_gain):
    """Build the 8 per-core input maps."""
    f32 = np.float32
    inv_freq = 1.0 / (ROPE_BASE ** (np.arange(0, HD, 2, dtype=f32) / HD))
    freqs = np.outer(np.arange(S, dtype=f32), inv_freq).astype(f32)
    c = np.cos(freqs).astype(f32)
    s = np.sin(freqs).astype(f32)
    cos2 = np.concatenate([c, c], axis=1)
    sin2 = np.concatenate([s, -s], axis=1)
    tri = np.triu(np.ones((P, P), dtype=f32))          # tri[k, q] = k <= q
    onesd = np.ones((P, P), dtype=f32)
    ident = np.eye(P, dtype=f32)

    in_maps = []
    for core in range(8):
        b, g = divmod(core, KVH)
        hs = g * NH * HD            # first q row for this group
        qg = (q_gain[g * NH:(g + 1) * NH].astype(f32) * (HD ** -0.5))
        in_maps.append({
            "xT": np.ascontiguousarray(x[b].T, dtype=f32),
            "wqT": np.ascontiguousarray(Wq[hs:hs + NH * HD].T, dtype=f32),
            "wkvT": np.ascontiguousarray(
                np.concatenate([Wk[g * HD:(g + 1) * HD], Wv[g * HD:(g + 1) * HD]],
                               axis=0).T, dtype=f32),
            "wpT": np.ascontiguousarray(Wproj.T[hs:hs + NH * HD], dtype=f32),
            "cos2": cos2, "sin2": sin2,
            "qg4": np.ascontiguousarray(np.broadcast_to(qg, (P, NH)), dtype=f32),
            "tri": tri, "onesd": onesd, "ident": ident,
        })
    return in_maps


def kernel(x, Wq, Wk, Wv, Wproj, q_gain):
    from concourse.bass_utils import run_bass_kernel_spmd

    x = np.asarray(x, dtype=np.float32)
    Wq = np.asarray(Wq, dtype=np.float32)
    Wk = np.asarray(Wk, dtype=np.float32)
    Wv = np.asarray(Wv, dtype=np.float32)
    Wproj = np.asarray(Wproj, dtype=np.float32)
    q_gain = np.asarray(q_gain, dtype=np.float32)

    if "nc" not in _CACHE:
        _CACHE["nc"] = _build_nc()
    nc = _CACHE["nc"]

    in_maps = _host_inputs(x, Wq, Wk, Wv, Wproj, q_gain)
    res = run_bass_kernel_spmd(nc, in_maps, core_ids=list(range(8)))

    out = np.zeros((B, S, D), dtype=np.float32)
    for core in range(8):
        b = core // KVH
        out[b] += res.results[core]["outT"].T
    return out



# revision 2
# speedup vs baseline: 1.0294x; 1.0294x over previous
"""Trainium2 Bass kernel for CausalSelfAttention (GQA + QK-RMSNorm + RoPE).

Problem shapes (hardcoded): B=2, S=2048, D=2048, H=16, KVH=4, HD=128.

Sharding: 8 cores = 2 batches x 4 kv-head groups. Core c handles batch
b = c // 4 and kv-group g = c % 4 (q-heads 4g..4g+3, kv head g).  Each core
computes its 4 heads end-to-end plus a partial output projection over its
512 columns of Wproj's input dim; the host sums the 4 partials per batch.

All matmul operands and all DMA'd tensors are bf16 (PSUM accumulation stays
fp32); this halves HBM traffic and removes the fp32r small-free-dim matmul
penalty on the diagonal attention tiles.

On-chip dataflow per core:
  Phase 1: stream xT (bf16), GEMM Q/K/V in [s, e] layout, QK-RMSNorm +
           RoPE + gain on DVE/ACT (outputs bf16), then one DMA-XBAR
           transpose per s-tile moves Q (all 4 heads at once) and K into
           [hd, s] layout -- no PE transposes and no PSUM->SBUF copies.
  Phase 2: causal attention per (q-block 512, head): scores computed
           TRANSPOSED [k, q] so the softmax denominator comes from a
           ones-matmul (which also broadcasts it across partitions); exp on
           ACT writes bf16; diagonal tiles masked with a triangular mask;
           matmul free dims rounded up to >=256; the denominator uses DVE
           running sums over groups of 4 k-tiles (diagonal included) so the
           ones-matmul runs once per group; PV matmul accumulates
           attnout^T [hd, q] over k-tiles in PSUM.
  Phase 3: partial out^T [e, s] = Wproj_slice^T-chunks @ y^T, bf16 staging,
           DMA to HBM via the gpsimd SWDGE path (keeps HWDGE free).
"""

import numpy as np

B, S, D = 2, 2048, 2048
H, KVH = 16, 4
HD = D // H            # 128
NH = H // KVH          # 4 heads per core
P = 128
ST = S // P            # 16 s-tiles
DT = D // P            # 16 d-tiles
FT = NH * HD // P      # 4 f-tiles (proj contraction per core)
QB = 512               # q-block width in phase 2
NQB = S // QB          # 4
SBW = 256              # phase-1 x DMA block width (s columns)
ROPE_BASE = 10000.0
EPS = 1e-6

_CACHE = {}


def _build_nc():
    from contextlib import ExitStack

    import concourse.mybir as mybir
    import concourse.tile as tile
    from concourse import bacc

    f32 = mybir.dt.float32
    bf16 = mybir.dt.bfloat16
    AF = mybir.ActivationFunctionType
    MUL = mybir.AluOpType.mult
    ADD = mybir.AluOpType.add

    nc = bacc.Bacc("TRN2", target_bir_lowering=False, debug=False, num_devices=8)

    xT = nc.dram_tensor("xT", [D, S], bf16, kind="ExternalInput").ap()
    wqT = nc.dram_tensor("wqT", [D, NH * HD], bf16, kind="ExternalInput").ap()
    wkvT = nc.dram_tensor("wkvT", [D, 2 * HD], bf16, kind="ExternalInput").ap()
    wpT = nc.dram_tensor("wpT", [NH * HD, D], bf16, kind="ExternalInput").ap()
    cs2 = nc.dram_tensor("cs2", [S, 2 * HD], bf16, kind="ExternalInput").ap()
    qg4 = nc.dram_tensor("qg4", [P, NH], f32, kind="ExternalInput").ap()
    tri = nc.dram_tensor("tri", [P, P], bf16, kind="ExternalInput").ap()
    onesd = nc.dram_tensor("onesd", [P, P], bf16, kind="ExternalInput").ap()
    outT = nc.dram_tensor("outT", [D, S], bf16, kind="ExternalOutput").ap()

    with tile.TileContext(nc) as tc:
        with ExitStack() as octx:
            const = octx.enter_context(tc.tile_pool(name="const", bufs=1))
            big = octx.enter_context(tc.tile_pool(name="big", bufs=1))

            # ---- persistent stores (all bf16) ----
            QT = big.tile([P, NH, S], bf16)   # q^T per head: [hd, h, s]
            KT = big.tile([P, S], bf16)       # k^T: [hd, s]
            VS = big.tile([P, ST, HD], bf16)  # v: [s-part, s-tile, hd]
            YT = big.tile([P, NH, S], bf16)   # attn out^T per head: [hd, h, s]

            # =========================== Phase 1 ===========================
            with ExitStack() as ctx1:
                wpool = ctx1.enter_context(tc.tile_pool(name="wpool", bufs=1))
                xpool = ctx1.enter_context(tc.tile_pool(name="xpool", bufs=2))
                stq = ctx1.enter_context(tc.tile_pool(name="stq", bufs=2))
                stk = ctx1.enter_context(tc.tile_pool(name="stk", bufs=2))
                sml = ctx1.enter_context(tc.tile_pool(name="sml", bufs=2))
                ps_q = ctx1.enter_context(
                    tc.tile_pool(name="ps_q", bufs=2, space="PSUM"))
                ps_kv = ctx1.enter_context(
                    tc.tile_pool(name="ps_kv", bufs=2, space="PSUM"))

                xTr = xT.rearrange("(dt p) s -> p dt s", p=P)
                wqTr = wqT.rearrange("(dt p) e -> p dt e", p=P)
                wkvTr = wkvT.rearrange("(dt p) e -> p dt e", p=P)

                WQ = wpool.tile([P, DT, NH * HD], bf16)
                WKV = wpool.tile([P, DT, 2 * HD], bf16)

                def load_xblk(sb):
                    t = xpool.tile([P, DT, SBW], bf16, tag="xblk", name="xblk")
                    for c in range(0, DT, 8):
                        nc.sync.dma_start(
                            t[:, c:c + 8, :],
                            xTr[:, c:c + 8, sb * SBW:(sb + 1) * SBW])
                    return t

                # startup order: small KV weights, first half of WQ, first
                # x block, rest of the weights, constants.
                nc.sync.dma_start(WKV[:], wkvTr[:])
                nc.sync.dma_start(WQ[:, 0:8, :], wqTr[:, 0:8, :])
                xblk_next = load_xblk(0)
                nc.sync.dma_start(WQ[:, 8:16, :], wqTr[:, 8:16, :])

                # ---- constants ----
                cs_t = const.tile([P, ST, 2 * HD], bf16)
                nc.sync.dma_start(cs_t[:], cs2.rearrange("(st p) c -> p st c", p=P))
                qg_t = const.tile([P, NH], f32)
                nc.sync.dma_start(qg_t[:], qg4)
                tri_t = const.tile([P, P], bf16)
                nc.sync.dma_start(tri_t[:], tri)
                ones_t = const.tile([P, P], bf16)
                nc.sync.dma_start(ones_t[:], onesd)
                eps_t = const.tile([P, 1], f32)
                nc.vector.memset(eps_t[:], EPS)

                for sb in range(S // SBW):
                    xblk = xblk_next
                    if sb + 1 < S // SBW:
                        xblk_next = load_xblk(sb + 1)
                    for jj in range(SBW // P):
                        st = sb * (SBW // P) + jj
                        xs = xblk[:, :, jj * P:(jj + 1) * P]

                        psq = ps_q.tile([P, NH * HD], f32)
                        for dt in range(DT):
                            nc.tensor.matmul(
                                psq[:], xs[:, dt],
                                WQ[:, dt],
                                start=(dt == 0), stop=(dt == DT - 1))
                        pskv = ps_kv.tile([P, 2 * HD], f32)
                        for dt in range(DT):
                            nc.tensor.matmul(
                                pskv[:], xs[:, dt],
                                WKV[:, dt],
                                start=(dt == 0), stop=(dt == DT - 1))

                        # V straight to its store (bf16)
                        nc.scalar.copy(VS[:, st], pskv[:, HD:2 * HD])

                        # -- Q rmsnorm + rope + gain --
                        psq3 = psq[:].rearrange("p (h c) -> p h c", h=NH)
                        ssq = sml.tile([P, NH], f32, tag="ssq")
                        sqscr = stq.tile([P, NH, HD], f32, tag="qsq")
                        for h in range(NH):
                            nc.scalar.activation(sqscr[:, h], psq3[:, h], AF.Square,
                                                 accum_out=ssq[:, h:h + 1])
                        msq = sml.tile([P, NH], f32, tag="msq")
                        nc.scalar.activation(msq[:], ssq[:], AF.Sqrt,
                                             bias=eps_t[:], scale=1.0 / HD)
                        rsq = sml.tile([P, NH], f32, tag="rsq")
                        nc.vector.reciprocal(rsq[:], msq[:])
                        rsg = sml.tile([P, NH], f32, tag="rsg")
                        nc.vector.tensor_tensor(rsg[:], rsq[:], qg_t[:], MUL)

                        qn = stq.tile([P, NH, HD], bf16, tag="qn")
                        nc.vector.tensor_tensor(
                            qn[:], psq3, rsg[:, :, None].to_broadcast([P, NH, HD]),
                            MUL)
                        cos_bc = cs_t[:, st:st + 1, 0:HD].to_broadcast([P, NH, HD])
                        qa = stq.tile([P, NH, HD], bf16, tag="qa")
                        nc.vector.tensor_tensor(qa[:], qn[:], cos_bc, MUL)
                        qb = stq.tile([P, NH, HD], bf16, tag="qb")
                        nc.vector.tensor_tensor(
                            qb[:, :, 0:HD // 2], qn[:, :, HD // 2:HD],
                            cs_t[:, st:st + 1, HD:HD + HD // 2].to_broadcast(
                                [P, NH, HD // 2]), MUL)
                        nc.vector.tensor_tensor(
                            qb[:, :, HD // 2:HD], qn[:, :, 0:HD // 2],
                            cs_t[:, st:st + 1, HD + HD // 2:2 * HD].to_broadcast(
                                [P, NH, HD // 2]), MUL)
                        qrot = stq.tile([P, NH, HD], bf16, tag="qr")
                        nc.vector.tensor_tensor(qrot[:], qa[:], qb[:], ADD)

                        # -- K rmsnorm + rope --
                        ssk = sml.tile([P, 1], f32, tag="ssk")
                        skscr = stk.tile([P, HD], f32, tag="ksq")
                        nc.scalar.activation(skscr[:], pskv[:, 0:HD], AF.Square,
                                             accum_out=ssk[:])
                        msk = sml.tile([P, 1], f32, tag="msk")
                        nc.scalar.activation(msk[:], ssk[:], AF.Sqrt,
                                             bias=eps_t[:], scale=1.0 / HD)
                        rsk = sml.tile([P, 1], f32, tag="rsk")
                        nc.vector.reciprocal(rsk[:], msk[:])

                        kn = stk.tile([P, HD], bf16, tag="kn")
                        nc.vector.tensor_tensor(
                            kn[:], pskv[:, 0:HD], rsk[:].to_broadcast([P, HD]), MUL)
                        ka = stk.tile([P, HD], bf16, tag="ka")
                        nc.gpsimd.tensor_tensor(ka[:], kn[:], cs_t[:, st, 0:HD], MUL)
                        kb = stk.tile([P, HD], bf16, tag="kb")
                        nc.gpsimd.tensor_tensor(
                            kb[:, 0:HD // 2], kn[:, HD // 2:HD],
                            cs_t[:, st, HD:HD + HD // 2], MUL)
                        nc.gpsimd.tensor_tensor(
                            kb[:, HD // 2:HD], kn[:, 0:HD // 2],
                            cs_t[:, st, HD + HD // 2:2 * HD], MUL)
                        krot = stk.tile([P, HD], bf16, tag="kr")
                        nc.gpsimd.tensor_tensor(krot[:], ka[:], kb[:], ADD)

                        # -- DMA-XBAR transposes into QT / KT --
                        nc.sync.dma_start_transpose(
                            QT[:, :, st * P:(st + 1) * P], qrot[:])
                        nc.sync.dma_start_transpose(
                            KT[:, st * P:(st + 1) * P], krot[:])

            # ======================= Phases 2 and 3 ========================
            with ExitStack() as ctx2:
                wp2 = ctx2.enter_context(tc.tile_pool(name="wp2", bufs=1))
                ostage = ctx2.enter_context(tc.tile_pool(name="ostage", bufs=4))

                WP = wp2.tile([P, FT, D], bf16)
                nc.sync.dma_start(WP[:], wpT.rearrange("(ft p) e -> p ft e", p=P))

                with ExitStack() as ctx2b:
                    expool = ctx2b.enter_context(
                        tc.tile_pool(name="expool", bufs=12))
                    recpool = ctx2b.enter_context(
                        tc.tile_pool(name="recpool", bufs=3))
                    ps_s = ctx2b.enter_context(
                        tc.tile_pool(name="ps_s", bufs=4, space="PSUM"))
                    ps_o = ctx2b.enter_context(
                        tc.tile_pool(name="ps_o", bufs=1, space="PSUM"))
                    ps_d = ctx2b.enter_context(
                        tc.tile_pool(name="ps_d", bufs=1, space="PSUM"))
                    ps_p3 = ctx2b.enter_context(
                        tc.tile_pool(name="ps_p3", bufs=2, space="PSUM"))

                    for qb in range(NQB):
                        for h in range(NH):
                            oT = ps_o.tile([P, QB], f32)
                            den = ps_d.tile([P, QB], f32)
                            nk = NH * qb + NH
                            grp_sum, grp_n = None, 0
                            for kt in range(nk):
                                j = kt - NH * qb  # >= 0 on diagonal tiles
                                q0 = P * j if j >= 0 else 0
                                # matmul start col, rounded so free dim >= 256
                                q0m = q0 if QB - q0 >= 256 else QB - 256
                                ps = ps_s.tile([P, QB], f32)
                                nc.tensor.matmul(
                                    ps[:, q0m:QB],
                                    KT[:, kt * P:(kt + 1) * P],
                                    QT[:, h, qb * QB + q0m:(qb + 1) * QB],
                                    start=True, stop=True)
                                ex = expool.tile([P, QB], bf16, tag="ex")
                                if j >= 1:
                                    nc.gpsimd.memset(
                                        ex[:, 0:q0].bitcast(mybir.dt.uint16), 0)
                                nc.scalar.activation(
                                    ex[:, q0:QB], ps[:, q0:QB], AF.Exp)
                                if j >= 0:
                                    nc.vector.tensor_tensor(
                                        ex[:, q0:q0 + P], ex[:, q0:q0 + P],
                                        tri_t[:], MUL)
                                nc.tensor.matmul(
                                    oT[:, q0m:QB], VS[:, kt], ex[:, q0m:QB],
                                    start=(kt == 0), stop=(kt == nk - 1))
                                # denominator: DVE running sums over groups of
                                # 4 k-tiles (diag tiles are zero-masked, so
                                # full-width adds are valid); one ones-matmul
                                # per group on PE.
                                if grp_sum is None:
                                    grp_sum, grp_n = ex, 1
                                else:
                                    ns = expool.tile([P, QB], bf16, tag="exs")
                                    nc.vector.tensor_tensor(
                                        ns[:], grp_sum[:], ex[:], ADD)
                                    grp_sum, grp_n = ns, grp_n + 1
                                if grp_n == 4 or kt == nk - 1:
                                    nc.tensor.matmul(
                                        den[:], ones_t[:], grp_sum[:],
                                        start=(kt < 4), stop=(kt == nk - 1))
                                    grp_sum, grp_n = None, 0
                            rec = recpool.tile([P, QB], f32, tag="rec")
                            nc.vector.reciprocal_approx_fast(rec[:], den[:])
                            nc.vector.tensor_tensor(
                                YT[:, h, qb * QB:(qb + 1) * QB], oT[:], rec[:],
                                MUL)

                        # phase-3 for this s-block (= qb) fills PE gaps
                        sb3 = qb
                        outTr = outT.rearrange("(et p) s -> p et s", p=P)
                        for et in range(0, DT, 2):
                            ob = ostage.tile([P, 2, QB], bf16)
                            for e in range(2):
                                po = ps_p3.tile([P, QB], f32)
                                for ft in range(FT):
                                    nc.tensor.matmul(
                                        po[:],
                                        WP[:, ft, (et + e) * P:(et + e + 1) * P],
                                        YT[:, ft, sb3 * QB:(sb3 + 1) * QB],
                                        start=(ft == 0), stop=(ft == FT - 1))
                                if e == 0:
                                    nc.scalar.copy(ob[:, e], po[:])
                                else:
                                    nc.vector.tensor_copy(ob[:, e], po[:])
                            nc.gpsimd.dma_start(
                                outTr[:, et:et + 2,
                                      sb3 * QB:(sb3 + 1) * QB], ob[:])

    nc.compile()
    return nc


def _host_inputs(x, Wq, Wk, Wv, Wproj, q_gain):
    """Build the 8 per-core input maps (bf16 on-device tensors)."""
    import ml_dtypes
    bf16 = ml_dtypes.bfloat16
    f32 = np.float32
    inv_freq = 1.0 / (ROPE_BASE ** (np.arange(0, HD, 2, dtype=f32) / HD))
    freqs = np.outer(np.arange(S, dtype=f32), inv_freq).astype(f32)
    c = np.cos(freqs).astype(f32)
    s = np.sin(freqs).astype(f32)
    cos2 = np.concatenate([c, c], axis=1)
    sin2 = np.concatenate([s, -s], axis=1)
    cs2 = np.ascontiguousarray(
        np.concatenate([cos2, sin2], axis=1).astype(bf16))
    tri = np.triu(np.ones((P, P), dtype=f32)).astype(bf16)  # tri[k,q]=k<=q
    onesd = np.ones((P, P), dtype=bf16)

    in_maps = []
    for core in range(8):
        b, g = divmod(core, KVH)
        hs = g * NH * HD            # first q row for this group
        qg = (q_gain[g * NH:(g + 1) * NH].astype(f32) * (HD ** -0.5))
        in_maps.append({
            "xT": np.ascontiguousarray(x[b].T.astype(bf16)),
            "wqT": np.ascontiguousarray(Wq[hs:hs + NH * HD].T.astype(bf16)),
            "wkvT": np.ascontiguousarray(
                np.concatenate([Wk[g * HD:(g + 1) * HD], Wv[g * HD:(g + 1) * HD]],
                               axis=0).T.astype(bf16)),
            "wpT": np.ascontiguousarray(Wproj.T[hs:hs + NH * HD].astype(bf16)),
            "cs2": cs2,
            "qg4": np.ascontiguousarray(np.broadcast_to(qg, (P, NH)), dtype=f32),
            "tri": tri, "onesd": onesd,
        })
    return in_maps


def kernel(x, Wq, Wk, Wv, Wproj, q_gain):
    from concourse.bass_utils import run_bass_kernel_spmd

    x = np.asarray(x, dtype=np.float32)
    Wq = np.asarray(Wq, dtype=np.float32)
    Wk = np.asarray(Wk, dtype=np.float32)
    Wv = np.asarray(Wv, dtype=np.float32)
    Wproj = np.asarray(Wproj, dtype=np.float32)
    q_gain = np.asarray(q_gain, dtype=np.float32)

    if "nc" not in _CACHE:
        _CACHE["nc"] = _build_nc()
    nc = _CACHE["nc"]

    in_maps = _host_inputs(x, Wq, Wk, Wv, Wproj, q_gain)
    res = run_bass_kernel_spmd(nc, in_maps, core_ids=list(range(8)))

    out = np.zeros((B, S, D), dtype=np.float32)
    for core in range(8):
        b = core // KVH
        out[b] += res.results[core]["outT"].T.astype(np.float32)
    return out


# revision 4
# speedup vs baseline: 1.0975x; 1.0662x over previous
"""Trainium2 Bass kernel for CausalSelfAttention (GQA + QK-RMSNorm + RoPE).

Problem shapes (hardcoded): B=2, S=2048, D=2048, H=16, KVH=4, HD=128.

Sharding: 8 cores = 2 batches x 4 kv-head groups. Core c handles batch
b = c // 4 and kv-group g = c % 4 (q-heads 4g..4g+3, kv head g).  Each core
computes its 4 heads end-to-end plus a partial output projection over its
512 columns of Wproj's input dim; the host sums the 4 partials per batch.

All matmul operands and all DMA'd tensors are bf16 (PSUM accumulation stays
fp32). One TileContext with no intermediate pool closes, so phase
boundaries carry no barriers. PSUM bank budget (8 banks):
  psA (3): phase-1 Q psum / phase-2 score tiles
  od "o" (2): phase-1 KV psum / phase-2 attn-out accumulator
  od "d" (1): softmax denominator accumulator
  p3  (2): output-projection accumulator

Dataflow per core:
  Phase 1: stream xT (bf16), GEMM Q/K/V in [s, e] layout, QK-RMSNorm +
           RoPE + gain on DVE/ACT (bf16 outputs), one DMA-XBAR transpose
           per s-tile for Q (all 4 heads) and K into [hd, s] layout.
  Phase 2: causal attention per (q-block 512, head): scores TRANSPOSED
           [k, q]; exp on ACT writes bf16; diag tiles masked with a
           triangular mask on GPSIMD; matmul free dims rounded up to
           >=256; denominator via in-place DVE running sums over groups of
           8 k-tiles + one ones-matmul per group; PV accumulates
           attnout^T [hd, q] in PSUM.
  Phase 3: partial out^T = WprojT-chunks @ y^T; the 16 column-tile units
           for q-block qb-1 are interleaved into the attention k-tile
           stream of q-block qb to fill PE gaps; bf16 staging; DMA to HBM
           via the gpsimd SWDGE path (keeps HWDGE free).
"""

import numpy as np

B, S, D = 2, 2048, 2048
H, KVH = 16, 4
HD = D // H            # 128
NH = H // KVH          # 4 heads per core
P = 128
ST = S // P            # 16 s-tiles
DT = D // P            # 16 d-tiles
FT = NH * HD // P      # 4 f-tiles (proj contraction per core)
QB = 512               # q-block width in phase 2
NQB = S // QB          # 4
SBW = 256              # phase-1 x DMA block width (s columns)
GRP = 8                # k-tiles per softmax-denominator group
ROPE_BASE = 10000.0
EPS = 1e-6

_CACHE = {}


def _build_nc():
    from collections import deque
    from contextlib import ExitStack

    import concourse.mybir as mybir
    import concourse.tile as tile
    from concourse import bacc

    f32 = mybir.dt.float32
    bf16 = mybir.dt.bfloat16
    AF = mybir.ActivationFunctionType
    MUL = mybir.AluOpType.mult
    ADD = mybir.AluOpType.add

    nc = bacc.Bacc("TRN2", target_bir_lowering=False, debug=False, num_devices=8)

    xT = nc.dram_tensor("xT", [D, S], bf16, kind="ExternalInput").ap()
    wqT = nc.dram_tensor("wqT", [D, NH * HD], bf16, kind="ExternalInput").ap()
    wkvT = nc.dram_tensor("wkvT", [D, 2 * HD], bf16, kind="ExternalInput").ap()
    wpT = nc.dram_tensor("wpT", [NH * HD, D], bf16, kind="ExternalInput").ap()
    cs2 = nc.dram_tensor("cs2", [S, 2 * HD], bf16, kind="ExternalInput").ap()
    qg4 = nc.dram_tensor("qg4", [P, NH], f32, kind="ExternalInput").ap()
    tri = nc.dram_tensor("tri", [P, P], bf16, kind="ExternalInput").ap()
    onesd = nc.dram_tensor("onesd", [P, P], bf16, kind="ExternalInput").ap()
    outT = nc.dram_tensor("outT", [D, S], bf16, kind="ExternalOutput").ap()

    with tile.TileContext(nc) as tc:
        with ExitStack() as octx:
            const = octx.enter_context(tc.tile_pool(name="const", bufs=1))
            big = octx.enter_context(tc.tile_pool(name="big", bufs=1))
            wpool = octx.enter_context(tc.tile_pool(name="wpool", bufs=1))
            xpool = octx.enter_context(tc.tile_pool(name="xpool", bufs=2))
            stq = octx.enter_context(tc.tile_pool(name="stq", bufs=2))
            stk = octx.enter_context(tc.tile_pool(name="stk", bufs=2))
            sml = octx.enter_context(tc.tile_pool(name="sml", bufs=2))
            expool = octx.enter_context(tc.tile_pool(name="expool", bufs=12))
            recpool = octx.enter_context(tc.tile_pool(name="recpool", bufs=3))
            ostage = octx.enter_context(tc.tile_pool(name="ostage", bufs=4))
            psA = octx.enter_context(
                tc.tile_pool(name="psA", bufs=3, space="PSUM"))
            od = octx.enter_context(
                tc.tile_pool(name="od", bufs=2, space="PSUM"))
            psD = octx.enter_context(
                tc.tile_pool(name="psD", bufs=1, space="PSUM"))
            ps_p3 = octx.enter_context(
                tc.tile_pool(name="ps_p3", bufs=2, space="PSUM"))

            # ---- persistent stores (all bf16) ----
            QT = big.tile([P, NH, S], bf16)   # q^T per head: [hd, h, s]
            KT = big.tile([P, S], bf16)       # k^T: [hd, s]
            VS = big.tile([P, ST, HD], bf16)  # v: [s-part, s-tile, hd]
            YT = big.tile([P, NH, S], bf16)   # attn out^T per head: [hd, h, s]

            xTr = xT.rearrange("(dt p) s -> p dt s", p=P)
            wqTr = wqT.rearrange("(dt p) e -> p dt e", p=P)
            wkvTr = wkvT.rearrange("(dt p) e -> p dt e", p=P)
            outTr = outT.rearrange("(et p) s -> p et s", p=P)

            WQ = wpool.tile([P, DT, NH * HD], bf16)
            WKV = wpool.tile([P, DT, 2 * HD], bf16)
            WP = wpool.tile([P, FT, D], bf16)

            def load_xblk(sb, nchunk=2):
                t = xpool.tile([P, DT, SBW], bf16, tag="xblk", name="xblk")
                step = DT // nchunk
                for c in range(0, DT, step):
                    nc.sync.dma_start(
                        t[:, c:c + step, :],
                        xTr[:, c:c + step, sb * SBW:(sb + 1) * SBW])
                return t

            # startup: interleave small WQ / x0 pieces so the first Q
            # matmuls can start after ~2 chunks; WKV + consts follow.
            xblk_next = xpool.tile([P, DT, SBW], bf16, tag="xblk", name="xblk")
            for c in range(0, DT, 4):
                nc.sync.dma_start(WQ[:, c:c + 4, :], wqTr[:, c:c + 4, :])
                nc.sync.dma_start(
                    xblk_next[:, c:c + 4, :], xTr[:, c:c + 4, 0:SBW])
            nc.sync.dma_start(WKV[:], wkvTr[:])

            # ---- constants ----
            cs_t = const.tile([P, ST, 2 * HD], bf16)
            nc.sync.dma_start(cs_t[:], cs2.rearrange("(st p) c -> p st c", p=P))
            qg_t = const.tile([P, NH], f32)
            nc.sync.dma_start(qg_t[:], qg4)
            tri_t = const.tile([P, P], bf16)
            nc.sync.dma_start(tri_t[:], tri)
            ones_t = const.tile([P, P], bf16)
            nc.sync.dma_start(ones_t[:], onesd)
            eps_t = const.tile([P, 1], f32)
            nc.vector.memset(eps_t[:], EPS)
            nc.sync.dma_start(WP[:], wpT.rearrange("(ft p) e -> p ft e", p=P))

            # =========================== Phase 1 ===========================
            for sb in range(S // SBW):
                xblk = xblk_next
                if sb + 1 < S // SBW:
                    xblk_next = load_xblk(sb + 1)
                for jj in range(SBW // P):
                    st = sb * (SBW // P) + jj
                    xs = xblk[:, :, jj * P:(jj + 1) * P]

                    psq = psA.tile([P, QB], f32, tag="ps")
                    for dt in range(DT):
                        nc.tensor.matmul(
                            psq[:], xs[:, dt],
                            WQ[:, dt],
                            start=(dt == 0), stop=(dt == DT - 1))
                    pskv = od.tile([P, QB], f32, tag="o")
                    for dt in range(DT):
                        nc.tensor.matmul(
                            pskv[:, 0:2 * HD], xs[:, dt],
                            WKV[:, dt],
                            start=(dt == 0), stop=(dt == DT - 1))

                    # V straight to its store (bf16)
                    nc.scalar.copy(VS[:, st], pskv[:, HD:2 * HD])

                    # -- Q rmsnorm + rope + gain --
                    psq3 = psq[:].rearrange("p (h c) -> p h c", h=NH)
                    ssq = sml.tile([P, NH], f32, tag="ssq")
                    sqscr = stq.tile([P, NH, HD], f32, tag="qsq")
                    for h in range(NH):
                        nc.scalar.activation(sqscr[:, h], psq3[:, h], AF.Square,
                                             accum_out=ssq[:, h:h + 1])
                    msq = sml.tile([P, NH], f32, tag="msq")
                    nc.scalar.activation(msq[:], ssq[:], AF.Sqrt,
                                         bias=eps_t[:], scale=1.0 / HD)
                    rsq = sml.tile([P, NH], f32, tag="rsq")
                    nc.vector.reciprocal(rsq[:], msq[:])
                    rsg = sml.tile([P, NH], f32, tag="rsg")
                    nc.vector.tensor_tensor(rsg[:], rsq[:], qg_t[:], MUL)

                    qn = stq.tile([P, NH, HD], bf16, tag="qn")
                    nc.vector.tensor_tensor(
                        qn[:], psq3, rsg[:, :, None].to_broadcast([P, NH, HD]),
                        MUL)
                    cos_bc = cs_t[:, st:st + 1, 0:HD].to_broadcast([P, NH, HD])
                    qa = stq.tile([P, NH, HD], bf16, tag="qa")
                    nc.vector.tensor_tensor(qa[:], qn[:], cos_bc, MUL)
                    qb_ = stq.tile([P, NH, HD], bf16, tag="qb")
                    nc.vector.tensor_tensor(
                        qb_[:, :, 0:HD // 2], qn[:, :, HD // 2:HD],
                        cs_t[:, st:st + 1, HD:HD + HD // 2].to_broadcast(
                            [P, NH, HD // 2]), MUL)
                    nc.vector.tensor_tensor(
                        qb_[:, :, HD // 2:HD], qn[:, :, 0:HD // 2],
                        cs_t[:, st:st + 1, HD + HD // 2:2 * HD].to_broadcast(
                            [P, NH, HD // 2]), MUL)
                    qrot = stq.tile([P, NH, HD], bf16, tag="qr")
                    nc.vector.tensor_tensor(qrot[:], qa[:], qb_[:], ADD)

                    # -- K rmsnorm + rope --
                    ssk = sml.tile([P, 1], f32, tag="ssk")
                    skscr = stk.tile([P, HD], f32, tag="ksq")
                    nc.scalar.activation(skscr[:], pskv[:, 0:HD], AF.Square,
                                         accum_out=ssk[:])
                    msk = sml.tile([P, 1], f32, tag="msk")
                    nc.scalar.activation(msk[:], ssk[:], AF.Sqrt,
                                         bias=eps_t[:], scale=1.0 / HD)
                    rsk = sml.tile([P, 1], f32, tag="rsk")
                    nc.vector.reciprocal(rsk[:], msk[:])

                    kn = stk.tile([P, HD], bf16, tag="kn")
                    nc.vector.tensor_tensor(
                        kn[:], pskv[:, 0:HD], rsk[:].to_broadcast([P, HD]), MUL)
                    ka = stk.tile([P, HD], bf16, tag="ka")
                    nc.gpsimd.tensor_tensor(ka[:], kn[:], cs_t[:, st, 0:HD], MUL)
                    kb = stk.tile([P, HD], bf16, tag="kb")
                    nc.gpsimd.tensor_tensor(
                        kb[:, 0:HD // 2], kn[:, HD // 2:HD],
                        cs_t[:, st, HD:HD + HD // 2], MUL)
                    nc.gpsimd.tensor_tensor(
                        kb[:, HD // 2:HD], kn[:, 0:HD // 2],
                        cs_t[:, st, HD + HD // 2:2 * HD], MUL)
                    krot = stk.tile([P, HD], bf16, tag="kr")
                    nc.gpsimd.tensor_tensor(krot[:], ka[:], kb[:], ADD)

                    # -- DMA-XBAR transposes into QT / KT --
                    nc.sync.dma_start_transpose(
                        QT[:, :, st * P:(st + 1) * P], qrot[:])
                    nc.sync.dma_start_transpose(
                        KT[:, st * P:(st + 1) * P], krot[:])

            # ======================= Phases 2 and 3 ========================
            # proj "units": one output column-tile (et) of a finished
            # q-block. Interleaved into the next q-block's attention.
            pending = deque()
            obcur = [None]

            def emit_proj_unit():
                if not pending:
                    return
                sb3, et = pending.popleft()
                po = ps_p3.tile([P, QB], f32, tag="po", name="po")
                for ft in range(FT):
                    nc.tensor.matmul(
                        po[:],
                        WP[:, ft, et * P:(et + 1) * P],
                        YT[:, ft, sb3 * QB:(sb3 + 1) * QB],
                        start=(ft == 0), stop=(ft == FT - 1))
                if et % 2 == 0:
                    obcur[0] = ostage.tile([P, 2, QB], bf16, tag="ob", name="ob")
                    nc.scalar.copy(obcur[0][:, 0], po[:])
                else:
                    nc.vector.tensor_copy(obcur[0][:, 1], po[:])
                    nc.gpsimd.dma_start(
                        outTr[:, et - 1:et + 1,
                              sb3 * QB:(sb3 + 1) * QB], obcur[0][:])

            for qb in range(NQB):
                nk = NH * qb + NH
                stride = max(1, (NH * nk) // (DT + 1))
                cnt = 0
                for h in range(NH):
                    oT = od.tile([P, QB], f32, tag="o")
                    den = psD.tile([P, QB], f32, tag="d")
                    grp_first, grp_n = None, 0
                    for kt in range(nk):
                        j = kt - NH * qb  # >= 0 on diagonal tiles
                        q0 = P * j if j >= 0 else 0
                        # matmul start col, rounded so free dim >= 256
                        q0m = q0 if QB - q0 >= 256 else QB - 256
                        ps = psA.tile([P, QB], f32, tag="ps")
                        nc.tensor.matmul(
                            ps[:, q0m:QB],
                            KT[:, kt * P:(kt + 1) * P],
                            QT[:, h, qb * QB + q0m:(qb + 1) * QB],
                            start=True, stop=True)
                        ex = expool.tile([P, QB], bf16, tag="ex")
                        if j >= 1:
                            nc.gpsimd.memset(
                                ex[:, 0:q0].bitcast(mybir.dt.uint16), 0)
                        nc.scalar.activation(
                            ex[:, q0:QB], ps[:, q0:QB], AF.Exp)
                        if j >= 0:
                            nc.gpsimd.tensor_tensor(
                                ex[:, q0:q0 + P], ex[:, q0:q0 + P],
                                tri_t[:], MUL)
                        nc.tensor.matmul(
                            oT[:, q0m:QB], VS[:, kt], ex[:, q0m:QB],
                            start=(kt == 0), stop=(kt == nk - 1))
                        # denominator: in-place DVE running sums over groups
                        # of GRP k-tiles; one ones-matmul per group.
                        if grp_first is None:
                            grp_first, grp_n = ex, 1
                        else:
                            nc.vector.tensor_tensor(
                                grp_first[:], grp_first[:], ex[:], ADD)
                            grp_n += 1
                        if grp_n == GRP or kt == nk - 1:
                            nc.tensor.matmul(
                                den[:], ones_t[:], grp_first[:],
                                start=(kt < GRP), stop=(kt == nk - 1))
                            grp_first, grp_n = None, 0
                        cnt += 1
                        if cnt % stride == 0:
                            emit_proj_unit()
                    rec = recpool.tile([P, QB], f32, tag="rec")
                    nc.vector.reciprocal_approx_fast(rec[:], den[:])
                    nc.vector.tensor_tensor(
                        YT[:, h, qb * QB:(qb + 1) * QB], oT[:], rec[:],
                        MUL)
                # flush any leftover units of the previous q-block, then
                # queue this q-block's 16 proj units.
                while pending:
                    emit_proj_unit()
                pending.extend((qb, et) for et in range(DT))
            while pending:
                emit_proj_unit()

    nc.compile()
    return nc


def _host_inputs(x, Wq, Wk, Wv, Wproj, q_gain):
    """Build the 8 per-core input maps (bf16 on-device tensors)."""
    import ml_dtypes
    bf16 = ml_dtypes.bfloat16
    f32 = np.float32
    inv_freq = 1.0 / (ROPE_BASE ** (np.arange(0, HD, 2, dtype=f32) / HD))
    freqs = np.outer(np.arange(S, dtype=f32), inv_freq).astype(f32)
    c = np.cos(freqs).astype(f32)
    s = np.sin(freqs).astype(f32)
    cos2 = np.concatenate([c, c], axis=1)
    sin2 = np.concatenate([s, -s], axis=1)
    cs2 = np.ascontiguousarray(
        np.concatenate([cos2, sin2], axis=1).astype(bf16))
    tri = np.triu(np.ones((P, P), dtype=f32)).astype(bf16)  # tri[k,q]=k<=q
    onesd = np.ones((P, P), dtype=bf16)

    in_maps = []
    for core in range(8):
        b, g = divmod(core, KVH)
        hs = g * NH * HD            # first q row for this group
        qg = (q_gain[g * NH:(g + 1) * NH].astype(f32) * (HD ** -0.5))
        in_maps.append({
            "xT": np.ascontiguousarray(x[b].T.astype(bf16)),
            "wqT": np.ascontiguousarray(Wq[hs:hs + NH * HD].T.astype(bf16)),
            "wkvT": np.ascontiguousarray(
                np.concatenate([Wk[g * HD:(g + 1) * HD], Wv[g * HD:(g + 1) * HD]],
                               axis=0).T.astype(bf16)),
            "wpT": np.ascontiguousarray(Wproj.T[hs:hs + NH * HD].astype(bf16)),
            "cs2": cs2,
            "qg4": np.ascontiguousarray(np.broadcast_to(qg, (P, NH)), dtype=f32),
            "tri": tri, "onesd": onesd,
        })
    return in_maps


def kernel(x, Wq, Wk, Wv, Wproj, q_gain):
    from concourse.bass_utils import run_bass_kernel_spmd

    x = np.asarray(x, dtype=np.float32)
    Wq = np.asarray(Wq, dtype=np.float32)
    Wk = np.asarray(Wk, dtype=np.float32)
    Wv = np.asarray(Wv, dtype=np.float32)
    Wproj = np.asarray(Wproj, dtype=np.float32)
    q_gain = np.asarray(q_gain, dtype=np.float32)

    if "nc" not in _CACHE:
        _CACHE["nc"] = _build_nc()
    nc = _CACHE["nc"]

    in_maps = _host_inputs(x, Wq, Wk, Wv, Wproj, q_gain)
    res = run_bass_kernel_spmd(nc, in_maps, core_ids=list(range(8)))

    out = np.zeros((B, S, D), dtype=np.float32)
    for core in range(8):
        b = core // KVH
        out[b] += res.results[core]["outT"].T.astype(np.float32)
    return out
